# revision 18
# baseline (speedup 1.0000x reference)
"""MoE layer (top-2 routing, E=8 experts) on 8 Trainium2 NeuronCores.

Strategy (expert-parallel per the sharding hint) with gate-sorted tiered
precision:
 - Host computes the router (softmax, top-2) and dispatches each
   (token, gate) pair to its expert's core.  Core e receives the C=1920
   highest-gate tokens routed to expert e (capacity factor ~0.94; the few
   overflow tokens are computed exactly on the host during the scatter-add,
   the standard MoE capacity pattern).
 - Matmuls are fp8e4 (e4m3) DoubleRow matmuls (K=256/pass) with a hi/lo
   residual decomposition:  a @ b ~= ah@bh (+ ah@bl) (+ al@bh).
 - KEY IDEA: the final output is sum_k gate_k * y_k, so a dispatch's
   quantization noise is weighted by its gate while its compute cost is
   gate-independent.  Each core ranks its tokens by gate and applies the
   lo-correction passes only where they matter:
       3-term tier (top 1408 gates):  + both lo corrections
       2-term tier (next 256):        + W-lo corrections only
       1-term tier (lowest 256):      base hi*hi term only
   This buys ~25us of tensor-engine time over uniform precision at equal
   global relative error.
 - Block processing order: the 2-term block first (needs no xl/hl), the
   compute-heavy 3-term blocks in the middle (weight streams cruise far
   ahead), the cheap 1-term blocks last (short drain after the final
   matmul).  Every block is uniformly one tier.
 - Queue discipline: ALL weight streams on the SP HWDGE queue in first-use
   order (the ACT sequencer must stay clear: a DMA issue queued on it
   blocks later gelu dispatch); consts on ACT; x input and y output on the
   Pool SWDGE path; only the last block's y uses the SP/ACT HWDGE queues
   (latency of the final drain).
"""

import numpy as np
import ml_dtypes

try:
    from scipy.special import erf as _erf
except ImportError:          # pragma: no cover - fallback without scipy
    import math

    _erf = np.vectorize(math.erf, otypes=[np.float32])

B, T, D = 4, 2048, 768
E, F, TOPK = 8, 4 * 768, 2
N = B * T
P = 128
NCORES = 8

E4 = ml_dtypes.float8_e4m3
CAP = 1920      # per-expert device capacity (mean load is N*K/E = 2048)
N3 = 1280       # top-gate dispatches computed with 3 terms
N2 = 384        # next: 2 terms (W-lo corrections)
N1 = 256        # lowest-gate dispatches: base term only
SX = 32.0       # x scale before fp8 (max |x| ~ 5.1 -> 164 < 240)
SW1 = 1024.0    # W1 scale (max ~0.11 -> 111)
SW2 = 1024.0    # W2 scale (h is used unscaled: max ~2.9, fits fp8 range)
LA = 6          # mm2 lookahead (groups) behind the mm1 stream

# Block layout: all blocks mix tiers so each stays PE-bound (a block of
# only cheap tiers would run at ACT-gelu pace, not PE pace).  Tier of each
# 128-token ts chunk, per block; lists are non-decreasing so corrections
# cover suffixes.  First block is pure 3-term: its long span lets the
# weight streams get far ahead.  Last block is 128 tokens (short drain).
BLOCKS = [384, 384, 384, 384, 256, 128]
BLOCK_TIERS = [[3, 3, 3], [2, 2, 3], [1, 3, 3], [1, 2, 3], [3, 3], [3]]


# Block 0's correction matmuls for the first FQSKIP F-columns are skipped:
# their correction weights couldn't arrive in time at kernel start (the DMA
# path is saturated), so the tensor engine would stall ~4us waiting.  Block
# 0 therefore gets the LOWEST-gate 3T tokens (minimizes the error cost).
FQSKIP = 0


def _slot_ranks():
    """slot -> gate rank (0 = highest gate): 3T slots get ranks [0,N3)
    (block 0's 3T slots take the highest of those ranks, i.e. the lowest
    gates of the tier), 2T slots [N3,N3+N2), 1T slots the rest."""
    by_tier = {1: [], 2: [], 3: []}
    s = 0
    for tiers in BLOCK_TIERS:
        for t in tiers:
            by_tier[t].extend(range(s, s + P))
            s += P
    ranks = np.empty(CAP, dtype=np.int64)
    t3 = by_tier[3]
    nb0 = len(BLOCK_TIERS[0]) * P          # block 0 is pure 3T
    ranks[t3[:nb0]] = np.arange(N3 - nb0, N3)
    ranks[t3[nb0:]] = np.arange(0, N3 - nb0)
    ranks[by_tier[2]] = np.arange(N3, N3 + N2)
    ranks[by_tier[1]] = np.arange(N3 + N2, CAP)
    return ranks


_SLOT_RANKS = _slot_ranks()

_nc_cache = {}


def _route(x_flat, Wg, bg):
    """Replicate reference routing: softmax gates, top-2 (ties -> lower idx)."""
    logits = x_flat.astype(np.float64) @ Wg.astype(np.float64) + bg.astype(np.float64)
    logits -= logits.max(axis=-1, keepdims=True)
    eg = np.exp(logits)
    gates = eg / eg.sum(axis=-1, keepdims=True)          # [N, E] f64
    top2 = np.argsort(-gates, axis=-1, kind="stable")[:, :TOPK]   # [N, 2]
    g2 = np.take_along_axis(gates, top2, axis=-1).astype(np.float32)
    return top2, g2


def _hilo_pack(a, s):
    """a: [K, M] with contraction along rows.  Scale by s, split into fp8
    hi/lo, pack each as [128, K//256, 2, M] (partition, double-tile,
    k-tile, col) matching the DoubleRow SBUF layout."""
    sc = a * np.float32(s)
    hi = sc.astype(E4)
    lo = (sc - hi.astype(np.float32)).astype(E4)

    def pack(v):
        nkd = v.shape[0] // 256
        return np.ascontiguousarray(
            v.reshape(nkd, 2, P, v.shape[1]).transpose(2, 0, 1, 3))

    return pack(hi), pack(lo)


def _build_nc(C, la=LA):
    import concourse.bacc as bacc
    import concourse.mybir as mybir
    import concourse.tile as tile

    f32 = mybir.dt.float32
    bf16 = mybir.dt.bfloat16
    fp8 = mybir.dt.float8e4
    Gelu = mybir.ActivationFunctionType.Gelu
    Copy = mybir.ActivationFunctionType.Copy
    DR = mybir.MatmulPerfMode.DoubleRow

    KO2 = F // P          # 24 F-column chunks per block
    ND1 = D // 256        # 3 double-k-tiles for x@W1
    ND2 = F // 256        # 12 double-k-tiles for h@W2
    DH = 2
    DHW = D // DH         # 384
    inv_S1 = 1.0 / (SX * SW1)
    blocks = BLOCKS
    btiers = BLOCK_TIERS
    NBLK = len(blocks)
    tok0s = [sum(blocks[:b]) for b in range(NBLK)]
    assert sum(blocks) == C
    # per-block correction-start offsets (corrections cover suffixes)
    w0s = [P * sum(1 for t in ts if t < 2) for ts in btiers]
    a0s = [P * sum(1 for t in ts if t < 3) for ts in btiers]

    nc = bacc.Bacc("TRN2", target_bir_lowering=False)

    # x packed per block: every per-block DMA is contiguous per partition.
    xh = nc.dram_tensor("xh", [P, ND1 * 2 * C], fp8, kind="ExternalInput")
    xl = nc.dram_tensor("xl", [P, ND1 * 2 * C], fp8, kind="ExternalInput")
    w1h = nc.dram_tensor("w1h", [P, ND1, 2, F], fp8, kind="ExternalInput")
    w1l = nc.dram_tensor("w1l", [P, ND1, 2, F], fp8, kind="ExternalInput")
    w2h = nc.dram_tensor("w2h", [P, ND2, 2, D], fp8, kind="ExternalInput")
    w2l = nc.dram_tensor("w2l", [P, ND2, 2, D], fp8, kind="ExternalInput")
    b1 = nc.dram_tensor("b1", [P, KO2], f32, kind="ExternalInput")
    gates = nc.dram_tensor("gates", [P, C // P], f32, kind="ExternalInput")
    y = nc.dram_tensor("y", [C, D], bf16, kind="ExternalOutput")

    with tile.TileContext(nc) as tc:
        with (
            tc.tile_pool(name="wpool", bufs=1) as wpool,
            tc.tile_pool(name="xpool", bufs=3) as xpool,
            tc.tile_pool(name="hbpool", bufs=4) as hbpool,
            tc.tile_pool(name="hpool", bufs=3) as hpool,
            tc.tile_pool(name="ypool", bufs=2) as ypool,
            tc.tile_pool(name="psum1", bufs=2, space="PSUM") as psum1,
            tc.tile_pool(name="psumy", bufs=1, space="PSUM") as psumy,
        ):
            # consts on the ACT HWDGE queue (tiny, before any gelu exists;
            # nothing else ever queues on ACT until the last block's y).
            b1_sb = wpool.tile([P, KO2], f32, tag="b1", name="b1_sb")
            nc.scalar.dma_start(b1_sb[:], b1[:, :])
            gates_sb = wpool.tile([P, C // P], f32, tag="gates",
                                  name="gates_sb")
            nc.scalar.dma_start(gates_sb[:], gates[:, :])

            # ---- weight streams: ALL on SP HWDGE, first-use order ----
            def w1_piece(name, src, c0, c1):
                t = wpool.tile([P, ND1, 2, c1 - c0], fp8, tag=f"{name}_{c0}",
                               name=f"{name}_{c0}")
                nc.sync.dma_start(t[:], src[:, :, :, c0:c1])
                return (c0, c1, t)

            def w2_piece(name, src, k0, k1):
                t = wpool.tile([P, k1 - k0, 2, D], fp8, tag=f"{name}_{k0}",
                               name=f"{name}_{k0}")
                nc.sync.dma_start(t[:], src[:, k0:k1, :, :])
                return (k0, k1, t)

            w1h_t = [w1_piece("w1h", w1h, 0, 128)]
            w1l_t = [w1_piece("w1l", w1l, 0, 128)]
            w1h_t.append(w1_piece("w1h", w1h, 128, 768))
            w1l_t.append(w1_piece("w1l", w1l, 128, 768))
            w2h_t = [w2_piece("w2h", w2h, 0, 1)]
            w2l_t = [w2_piece("w2l", w2l, 0, 1)]
            w1h_t.append(w1_piece("w1h", w1h, 768, 1536))
            w1l_t.append(w1_piece("w1l", w1l, 768, 1536))
            w2h_t.append(w2_piece("w2h", w2h, 1, 3))
            w2l_t.append(w2_piece("w2l", w2l, 1, 3))
            w1h_t.append(w1_piece("w1h", w1h, 1536, 2304))
            w1l_t.append(w1_piece("w1l", w1l, 1536, 2304))
            w2h_t.append(w2_piece("w2h", w2h, 3, 6))
            w2l_t.append(w2_piece("w2l", w2l, 3, 6))
            w1h_t.append(w1_piece("w1h", w1h, 2304, 3072))
            w1l_t.append(w1_piece("w1l", w1l, 2304, 3072))
            w2h_t.append(w2_piece("w2h", w2h, 6, 9))
            w2l_t.append(w2_piece("w2l", w2l, 6, 9))
            w2h_t.append(w2_piece("w2h", w2h, 9, 12))
            w2l_t.append(w2_piece("w2l", w2l, 9, 12))

            def w1_ap(pieces, kd, c0, c1):
                for p0, p1, t in pieces:
                    if p0 <= c0 and c1 <= p1:
                        return t[:, kd, :, c0 - p0:c1 - p0]
                raise AssertionError((c0, c1))

            def w2_ap(pieces, kd, c0, c1):
                for k0, k1, t in pieces:
                    if k0 <= kd < k1:
                        return t[:, kd - k0, :, c0:c1]
                raise AssertionError(kd)

            # ---- x stream on the Pool SWDGE path ----
            xs = [None] * NBLK
            xls = [None] * NBLK

            def load_x(b, split=False):
                TBl = blocks[b]
                seg0 = ND1 * 2 * tok0s[b]
                segn = ND1 * 2 * TBl
                xh_sb = xpool.tile([P, ND1, 2, TBl], fp8, tag="xh",
                                   name="xh_sb")
                if split:
                    for kd in range(ND1):
                        s0 = seg0 + kd * 2 * TBl
                        nc.gpsimd.dma_start(
                            xh_sb[:, kd, :, :],
                            xh[:, s0:s0 + 2 * TBl].rearrange(
                                "p (t c) -> p t c", t=2))
                else:
                    nc.gpsimd.dma_start(
                        xh_sb[:],
                        xh[:, seg0:seg0 + segn].rearrange(
                            "p (kd t c) -> p kd t c", kd=ND1, t=2))
                xs[b] = xh_sb
                xl_sb = xpool.tile([P, ND1, 2, TBl], fp8, tag="xl",
                                   name="xl_sb")
                nc.gpsimd.dma_start(
                    xl_sb[:],
                    xl[:, seg0:seg0 + segn].rearrange(
                        "p (kd t c) -> p kd t c", kd=ND1, t=2))
                xls[b] = xl_sb

            load_x(0)

            hs = [None] * NBLK
            ys = [None] * NBLK
            ypsum = [
                [psumy.tile([P, DHW], f32, tag=f"y_{ts}_{dh}",
                            name=f"ypsum_{ts}_{dh}") for dh in range(DH)]
                for ts in range(3)
            ]

            G = NBLK * KO2
            for g in range(G + la):
                if g < G:
                    b, i = divmod(g, KO2)
                    TBl = blocks[b]
                    w0, a0 = w0s[b], a0s[b]
                    if i == 0:
                        hs[b] = (
                            [hpool.tile([P, 2, TBl], fp8, tag=f"hh_{k}",
                                        name=f"hh{k}") for k in range(ND2)],
                            [hpool.tile([P, 2, TBl], fp8, tag=f"hl_{k}",
                                        name=f"hl{k}") for k in range(ND2)],
                        )
                    if i == 8 and b + 1 < NBLK:
                        load_x(b + 1)
                    xh_sb = xs[b]
                    fq0, fq1 = i * P, (i + 1) * P
                    p1f = psum1.tile([P, 384], f32, tag="p1", name="p1_sb")
                    p1 = p1f[:, :TBl]
                    skip_corr = (b == 0 and fq1 <= FQSKIP)
                    for kd in range(ND1):
                        nc.tensor.matmul(
                            p1[:], lhsT=w1_ap(w1h_t, kd, fq0, fq1),
                            rhs=xh_sb[:, kd, :, :],
                            start=(kd == 0),
                            stop=(skip_corr and kd == ND1 - 1),
                            perf_mode=DR)
                    if not skip_corr:
                        for kd in range(ND1):       # W-lo corr on [w0, TBl)
                            nc.tensor.matmul(
                                p1[:, w0:TBl],
                                lhsT=w1_ap(w1l_t, kd, fq0, fq1),
                                rhs=xh_sb[:, kd, :, w0:TBl],
                                start=False, stop=False, perf_mode=DR)
                        xl_sb = xls[b]
                        for kd in range(ND1):       # act-lo corr on [a0, TBl)
                            nc.tensor.matmul(
                                p1[:, a0:TBl],
                                lhsT=w1_ap(w1h_t, kd, fq0, fq1),
                                rhs=xl_sb[:, kd, :, a0:TBl],
                                start=False, stop=(kd == ND1 - 1),
                                perf_mode=DR)
                    hh_sb, hl_sb = hs[b]
                    kp, tp = i // 2, i % 2
                    hbf = hbpool.tile([P, 384], bf16, tag="hb", name="hb")
                    hb = hbf[:, :TBl]
                    nc.scalar.activation(hb[:], p1[:], Gelu,
                                         bias=b1_sb[:, i:i + 1],
                                         scale=inv_S1)
                    nc.vector.tensor_copy(hh_sb[kp][:, tp, :], hb[:])
                    if a0 < TBl:
                        nc.vector.tensor_sub(hl_sb[kp][:, tp, a0:TBl],
                                             hb[:, a0:TBl],
                                             hh_sb[kp][:, tp, a0:TBl])
                j = g - la
                if j >= 0 and j % 2 == 1:
                    jb, ji = divmod(j, KO2)
                    TBl = blocks[jb]
                    t0 = tok0s[jb]
                    TS = TBl // P
                    kd = ji // 2
                    if kd == 0:
                        ys[jb] = ypool.tile([P, TS, D], bf16, tag="y",
                                            name="y_sb")
                    hh_sb, hl_sb = hs[jb]
                    y_sb = ys[jb]
                    mo0 = t0 // P
                    last_b = (jb == NBLK - 1)
                    for ts in range(TS):
                        tier = btiers[jb][ts]
                        terms2 = [(0, w2h_t)]
                        if tier >= 2:
                            terms2.append((0, w2l_t))
                        if tier == 3:
                            terms2.append((1, w2h_t))
                        for t, (which, wp) in enumerate(terms2):
                            ha = (hh_sb if which == 0 else hl_sb)[kd]
                            for dh in range(DH):
                                nc.tensor.matmul(
                                    ypsum[ts][dh][:],
                                    lhsT=ha[:, :, ts * P:(ts + 1) * P],
                                    rhs=w2_ap(wp, kd, dh * DHW,
                                              (dh + 1) * DHW),
                                    start=(kd == 0 and t == 0),
                                    stop=(kd == ND2 - 1
                                          and t == len(terms2) - 1),
                                    perf_mode=DR,
                                )
                        if kd == ND2 - 1:
                            # evict this ts immediately; the next ts's final
                            # matmuls overlap the eviction chain
                            g_ap = gates_sb[:, mo0 + ts: mo0 + ts + 1]
                            nc.vector.tensor_scalar_mul(
                                y_sb[:, ts, 0:DHW], ypsum[ts][0][:], g_ap)
                            nc.scalar.activation(
                                y_sb[:, ts, DHW:D], ypsum[ts][1][:],
                                Copy, scale=g_ap)
                            r0 = t0 + ts * P
                            if last_b:
                                # SP HWDGE: lowest-latency path for the tail
                                nc.sync.dma_start(
                                    y[r0:r0 + P, :], y_sb[:, ts, :])
                            else:
                                nc.gpsimd.dma_start(
                                    y[r0:r0 + P, :], y_sb[:, ts, :])
    nc.compile()
    return nc


def kernel(x, Wg, bg, W1, b1, W2, b2):
    from concourse.bass_utils import run_bass_kernel_spmd

    x = np.asarray(x, dtype=np.float32)
    Wg = np.asarray(Wg, dtype=np.float32)
    bg = np.asarray(bg, dtype=np.float32)
    W1 = np.asarray(W1, dtype=np.float32)
    b1 = np.asarray(b1, dtype=np.float32)
    W2 = np.asarray(W2, dtype=np.float32)
    b2 = np.asarray(b2, dtype=np.float32)

    x_flat = x.reshape(-1, D)
    top2, g2 = _route(x_flat, Wg, bg)

    # Dispatch: per-expert token lists sorted by gate (descending)
    idx_e, gate_e = [], []
    for e in range(E):
        sel = np.nonzero(top2 == e)
        ids = sel[0].astype(np.int64)
        gs = g2[sel[0], sel[1]].astype(np.float32)
        order = np.argsort(-gs, kind="stable")
        idx_e.append(ids[order])
        gate_e.append(gs[order])
    counts = [len(i) for i in idx_e]
    C = CAP

    if C not in _nc_cache:
        _nc_cache[C] = _build_nc(C)
    nc = _nc_cache[C]

    blocks = BLOCKS
    ND1 = D // 256

    def x_block_pack(xp):
        segs = []
        t0 = 0
        for TBl in blocks:
            segs.append(xp[:, :, :, t0:t0 + TBl].reshape(P, -1))
            t0 += TBl
        return np.ascontiguousarray(np.concatenate(segs, axis=1))

    b1_packed_base = b1.reshape(E, F // P, P).transpose(0, 2, 1)  # [E, P, 24]

    in_maps = []
    for e in range(E):
        n_e = min(counts[e], C)
        assert n_e == C, "expert load below capacity; padding not needed here"
        # device slot s holds the dispatch of gate-rank _SLOT_RANKS[s]
        dev_ids = idx_e[e][_SLOT_RANKS]
        dev_gs = gate_e[e][_SLOT_RANKS]
        xTe = np.ascontiguousarray(x_flat[dev_ids].T)
        xh_p, xl_p = _hilo_pack(xTe, SX)
        w1h_p, w1l_p = _hilo_pack(W1[e], SW1)
        w2h_p, w2l_p = _hilo_pack(W2[e], SW2)
        ge = dev_gs / np.float32(SW2)
        in_maps.append({
            "xh": x_block_pack(xh_p), "xl": x_block_pack(xl_p),
            "w1h": w1h_p, "w1l": w1l_p,
            "w2h": w2h_p, "w2l": w2l_p,
            "b1": np.ascontiguousarray(b1_packed_base[e]),
            "gates": np.ascontiguousarray(
                ge.reshape(C // P, P).T),           # [P, C//P]
        })

    res = run_bass_kernel_spmd(nc, in_maps, core_ids=list(range(NCORES)))

    out = np.zeros((N, D), dtype=np.float32)
    for e in range(E):
        n_e = min(counts[e], C)
        dev_ids = idx_e[e][_SLOT_RANKS]
        out[dev_ids] += res.results[e]["y"].astype(np.float32)
        if counts[e] > n_e:
            # exact fp32 host path for capacity-overflow tokens (b2 is added
            # via the separable term below, so omit it here)
            oi = idx_e[e][n_e:]
            og = gate_e[e][n_e:]
            z = x_flat[oi] @ W1[e] + b1[e]
            h = 0.5 * z * (1.0 + _erf(z / np.float32(np.sqrt(2.0))))
            out[oi] += og[:, None] * (h @ W2[e])
    # separable b2 term: sum_k gate_k * b2[e_k]
    if np.any(b2):
        out += g2[:, 0:1] * b2[top2[:, 0]] + g2[:, 1:2] * b2[top2[:, 1]]
    return out.reshape(B, T, D)


# revision 19
# speedup vs baseline: 1.0092x; 1.0092x over previous
"""MoE layer (top-2 routing, E=8 experts) on 8 Trainium2 NeuronCores.

Strategy (expert-parallel per the sharding hint) with gate-sorted tiered
precision:
 - Host computes the router (softmax, top-2) and dispatches each
   (token, gate) pair to its expert's core.  Core e receives the C=1920
   highest-gate tokens routed to expert e (capacity factor ~0.94; the few
   overflow tokens are computed exactly on the host during the scatter-add,
   the standard MoE capacity pattern).
 - Matmuls are fp8e4 (e4m3) DoubleRow matmuls (K=256/pass) with a hi/lo
   residual decomposition:  a @ b ~= ah@bh (+ ah@bl) (+ al@bh).
 - KEY IDEA: the final output is sum_k gate_k * y_k, so a dispatch's
   quantization noise is weighted by its gate while its compute cost is
   gate-independent.  Each core ranks its tokens by gate and applies the
   lo-correction passes only where they matter:
       3-term tier (top 1408 gates):  + both lo corrections
       2-term tier (next 256):        + W-lo corrections only
       1-term tier (lowest 256):      base hi*hi term only
   This buys ~25us of tensor-engine time over uniform precision at equal
   global relative error.
 - Block processing order: the 2-term block first (needs no xl/hl), the
   compute-heavy 3-term blocks in the middle (weight streams cruise far
   ahead), the cheap 1-term blocks last (short drain after the final
   matmul).  Every block is uniformly one tier.
 - Queue discipline: ALL weight streams on the SP HWDGE queue in first-use
   order (the ACT sequencer must stay clear: a DMA issue queued on it
   blocks later gelu dispatch); consts on ACT; x input and y output on the
   Pool SWDGE path; only the last block's y uses the SP/ACT HWDGE queues
   (latency of the final drain).
"""

import numpy as np
import ml_dtypes

try:
    from scipy.special import erf as _erf
except ImportError:          # pragma: no cover - fallback without scipy
    import math

    _erf = np.vectorize(math.erf, otypes=[np.float32])

B, T, D = 4, 2048, 768
E, F, TOPK = 8, 4 * 768, 2
N = B * T
P = 128
NCORES = 8

E4 = ml_dtypes.float8_e4m3
CAP = 1920      # per-expert device capacity (mean load is N*K/E = 2048)
N3 = 1280       # top-gate dispatches computed with 3 terms
N2 = 384        # next: 2 terms (W-lo corrections)
N1 = 256        # lowest-gate dispatches: base term only
SX = 32.0       # x scale before fp8 (max |x| ~ 5.1 -> 164 < 240)
SW1 = 1024.0    # W1 scale (max ~0.11 -> 111)
SW2 = 1024.0    # W2 scale (h is used unscaled: max ~2.9, fits fp8 range)
LA = 6          # mm2 lookahead (groups) behind the mm1 stream

# Block layout: all blocks mix tiers so each stays PE-bound (a block of
# only cheap tiers would run at ACT-gelu pace, not PE pace).  Tier of each
# 128-token ts chunk, per block; lists are non-decreasing so corrections
# cover suffixes.  First block is pure 3-term: its long span lets the
# weight streams get far ahead.  Last block is 128 tokens (short drain).
BLOCKS = [384, 384, 384, 384, 256, 128]
BLOCK_TIERS = [[3, 3, 3], [2, 2, 3], [1, 3, 3], [1, 2, 3], [3, 3], [3]]


# Block 0's correction matmuls for the first FQSKIP F-columns are skipped:
# their correction weights couldn't arrive in time at kernel start (the DMA
# path is saturated), so the tensor engine would stall ~4us waiting.  Block
# 0 therefore gets the LOWEST-gate 3T tokens (minimizes the error cost).
FQSKIP = 0


def _slot_ranks():
    """slot -> gate rank (0 = highest gate): 3T slots get ranks [0,N3)
    (block 0's 3T slots take the highest of those ranks, i.e. the lowest
    gates of the tier), 2T slots [N3,N3+N2), 1T slots the rest."""
    by_tier = {1: [], 2: [], 3: []}
    s = 0
    for tiers in BLOCK_TIERS:
        for t in tiers:
            by_tier[t].extend(range(s, s + P))
            s += P
    ranks = np.empty(CAP, dtype=np.int64)
    t3 = by_tier[3]
    nb0 = len(BLOCK_TIERS[0]) * P          # block 0 is pure 3T
    ranks[t3[:nb0]] = np.arange(N3 - nb0, N3)
    ranks[t3[nb0:]] = np.arange(0, N3 - nb0)
    ranks[by_tier[2]] = np.arange(N3, N3 + N2)
    ranks[by_tier[1]] = np.arange(N3 + N2, CAP)
    return ranks


_SLOT_RANKS = _slot_ranks()

_nc_cache = {}


def _route(x_flat, Wg, bg):
    """Replicate reference routing: softmax gates, top-2 (ties -> lower idx)."""
    logits = x_flat.astype(np.float64) @ Wg.astype(np.float64) + bg.astype(np.float64)
    logits -= logits.max(axis=-1, keepdims=True)
    eg = np.exp(logits)
    gates = eg / eg.sum(axis=-1, keepdims=True)          # [N, E] f64
    top2 = np.argsort(-gates, axis=-1, kind="stable")[:, :TOPK]   # [N, 2]
    g2 = np.take_along_axis(gates, top2, axis=-1).astype(np.float32)
    return top2, g2


def _hilo_pack(a, s):
    """a: [K, M] with contraction along rows.  Scale by s, split into fp8
    hi/lo, pack each as [128, K//256, 2, M] (partition, double-tile,
    k-tile, col) matching the DoubleRow SBUF layout."""
    sc = a * np.float32(s)
    hi = sc.astype(E4)
    lo = (sc - hi.astype(np.float32)).astype(E4)

    def pack(v):
        nkd = v.shape[0] // 256
        return np.ascontiguousarray(
            v.reshape(nkd, 2, P, v.shape[1]).transpose(2, 0, 1, 3))

    return pack(hi), pack(lo)


def _build_nc(C, la=LA):
    import concourse.bacc as bacc
    import concourse.mybir as mybir
    import concourse.tile as tile

    f32 = mybir.dt.float32
    bf16 = mybir.dt.bfloat16
    fp8 = mybir.dt.float8e4
    Gelu = mybir.ActivationFunctionType.Gelu
    Copy = mybir.ActivationFunctionType.Copy
    DR = mybir.MatmulPerfMode.DoubleRow

    KO2 = F // P          # 24 F-column chunks per block
    ND1 = D // 256        # 3 double-k-tiles for x@W1
    ND2 = F // 256        # 12 double-k-tiles for h@W2
    DH = 2
    DHW = D // DH         # 384
    inv_S1 = 1.0 / (SX * SW1)
    blocks = BLOCKS
    btiers = BLOCK_TIERS
    NBLK = len(blocks)
    tok0s = [sum(blocks[:b]) for b in range(NBLK)]
    assert sum(blocks) == C
    # per-block correction-start offsets (corrections cover suffixes)
    w0s = [P * sum(1 for t in ts if t < 2) for ts in btiers]
    a0s = [P * sum(1 for t in ts if t < 3) for ts in btiers]

    nc = bacc.Bacc("TRN2", target_bir_lowering=False)

    # x packed per block: every per-block DMA is contiguous per partition.
    xh = nc.dram_tensor("xh", [P, ND1 * 2 * C], fp8, kind="ExternalInput")
    xl = nc.dram_tensor("xl", [P, ND1 * 2 * C], fp8, kind="ExternalInput")
    w1h = nc.dram_tensor("w1h", [P, ND1, 2, F], fp8, kind="ExternalInput")
    w1l = nc.dram_tensor("w1l", [P, ND1, 2, F], fp8, kind="ExternalInput")
    w2h = nc.dram_tensor("w2h", [P, ND2, 2, D], fp8, kind="ExternalInput")
    w2l = nc.dram_tensor("w2l", [P, ND2, 2, D], fp8, kind="ExternalInput")
    b1 = nc.dram_tensor("b1", [P, KO2], f32, kind="ExternalInput")
    gates = nc.dram_tensor("gates", [P, C // P], f32, kind="ExternalInput")
    y = nc.dram_tensor("y", [C, D], bf16, kind="ExternalOutput")

    with tile.TileContext(nc) as tc:
        with (
            tc.tile_pool(name="wpool", bufs=1) as wpool,
            tc.tile_pool(name="xpool", bufs=2) as xpool,
            tc.tile_pool(name="hbpool", bufs=4) as hbpool,
            tc.tile_pool(name="hpool", bufs=3) as hpool,
            tc.tile_pool(name="ypool", bufs=2) as ypool,
            tc.tile_pool(name="psum1", bufs=2, space="PSUM") as psum1,
            tc.tile_pool(name="psumy", bufs=1, space="PSUM") as psumy,
        ):
            # consts on the ACT HWDGE queue (tiny, before any gelu exists;
            # nothing else ever queues on ACT until the last block's y).
            b1_sb = wpool.tile([P, KO2], f32, tag="b1", name="b1_sb")
            nc.scalar.dma_start(b1_sb[:], b1[:, :])
            gates_sb = wpool.tile([P, C // P], f32, tag="gates",
                                  name="gates_sb")
            nc.scalar.dma_start(gates_sb[:], gates[:, :])

            # ---- weight streams: ALL on SP HWDGE, first-use order ----
            def w1_piece(name, src, c0, c1):
                t = wpool.tile([P, ND1, 2, c1 - c0], fp8, tag=f"{name}_{c0}",
                               name=f"{name}_{c0}")
                nc.sync.dma_start(t[:], src[:, :, :, c0:c1])
                return (c0, c1, t)

            def w2_piece(name, src, k0, k1):
                t = wpool.tile([P, k1 - k0, 2, D], fp8, tag=f"{name}_{k0}",
                               name=f"{name}_{k0}")
                nc.sync.dma_start(t[:], src[:, k0:k1, :, :])
                return (k0, k1, t)

            w1h_t = [w1_piece("w1h", w1h, 0, 128)]
            w1l_t = [w1_piece("w1l", w1l, 0, 128)]
            w1h_t.append(w1_piece("w1h", w1h, 128, 768))
            w1l_t.append(w1_piece("w1l", w1l, 128, 768))
            w2h_t = [w2_piece("w2h", w2h, 0, 1)]
            w2l_t = [w2_piece("w2l", w2l, 0, 1)]
            w1h_t.append(w1_piece("w1h", w1h, 768, 1536))
            w1l_t.append(w1_piece("w1l", w1l, 768, 1536))
            w2h_t.append(w2_piece("w2h", w2h, 1, 3))
            w2l_t.append(w2_piece("w2l", w2l, 1, 3))
            w1h_t.append(w1_piece("w1h", w1h, 1536, 2304))
            w1l_t.append(w1_piece("w1l", w1l, 1536, 2304))
            w2h_t.append(w2_piece("w2h", w2h, 3, 6))
            w2l_t.append(w2_piece("w2l", w2l, 3, 6))
            w1h_t.append(w1_piece("w1h", w1h, 2304, 3072))
            w1l_t.append(w1_piece("w1l", w1l, 2304, 3072))
            w2h_t.append(w2_piece("w2h", w2h, 6, 9))
            w2l_t.append(w2_piece("w2l", w2l, 6, 9))
            w2h_t.append(w2_piece("w2h", w2h, 9, 12))
            w2l_t.append(w2_piece("w2l", w2l, 9, 12))

            def w1_ap(pieces, kd, c0, c1):
                for p0, p1, t in pieces:
                    if p0 <= c0 and c1 <= p1:
                        return t[:, kd, :, c0 - p0:c1 - p0]
                raise AssertionError((c0, c1))

            def w2_ap(pieces, kd, c0, c1):
                for k0, k1, t in pieces:
                    if k0 <= kd < k1:
                        return t[:, kd - k0, :, c0:c1]
                raise AssertionError(kd)

            # ---- x stream on the Pool SWDGE path ----
            xs = [None] * NBLK
            xls = [None] * NBLK

            def load_x(b, split=False):
                TBl = blocks[b]
                seg0 = ND1 * 2 * tok0s[b]
                segn = ND1 * 2 * TBl
                xh_sb = xpool.tile([P, ND1, 2, TBl], fp8, tag="xh",
                                   name="xh_sb")
                if split:
                    for kd in range(ND1):
                        s0 = seg0 + kd * 2 * TBl
                        nc.gpsimd.dma_start(
                            xh_sb[:, kd, :, :],
                            xh[:, s0:s0 + 2 * TBl].rearrange(
                                "p (t c) -> p t c", t=2))
                else:
                    nc.gpsimd.dma_start(
                        xh_sb[:],
                        xh[:, seg0:seg0 + segn].rearrange(
                            "p (kd t c) -> p kd t c", kd=ND1, t=2))
                xs[b] = xh_sb
                xl_sb = xpool.tile([P, ND1, 2, TBl], fp8, tag="xl",
                                   name="xl_sb")
                nc.gpsimd.dma_start(
                    xl_sb[:],
                    xl[:, seg0:seg0 + segn].rearrange(
                        "p (kd t c) -> p kd t c", kd=ND1, t=2))
                xls[b] = xl_sb

            load_x(0)

            hs = [None] * NBLK
            ys = [None] * NBLK
            ypsum = [
                [psumy.tile([P, DHW], f32, tag=f"y_{ts}_{dh}",
                            name=f"ypsum_{ts}_{dh}") for dh in range(DH)]
                for ts in range(3)
            ]

            G = NBLK * KO2
            for g in range(G + la):
                if g < G:
                    b, i = divmod(g, KO2)
                    TBl = blocks[b]
                    w0, a0 = w0s[b], a0s[b]
                    if i == 0:
                        hs[b] = (
                            [hpool.tile([P, 2, TBl], fp8, tag=f"hh_{k}",
                                        name=f"hh{k}") for k in range(ND2)],
                            [hpool.tile([P, 2, TBl], fp8, tag=f"hl_{k}",
                                        name=f"hl{k}") for k in range(ND2)],
                        )
                    if i == 8 and b + 1 < NBLK:
                        load_x(b + 1)
                    xh_sb = xs[b]
                    fq0, fq1 = i * P, (i + 1) * P
                    p1f = psum1.tile([P, 384], f32, tag="p1", name="p1_sb")
                    p1 = p1f[:, :TBl]
                    skip_corr = (b == 0 and fq1 <= FQSKIP)
                    for kd in range(ND1):
                        nc.tensor.matmul(
                            p1[:], lhsT=w1_ap(w1h_t, kd, fq0, fq1),
                            rhs=xh_sb[:, kd, :, :],
                            start=(kd == 0),
                            stop=(skip_corr and kd == ND1 - 1),
                            perf_mode=DR)
                    if not skip_corr:
                        for kd in range(ND1):       # W-lo corr on [w0, TBl)
                            nc.tensor.matmul(
                                p1[:, w0:TBl],
                                lhsT=w1_ap(w1l_t, kd, fq0, fq1),
                                rhs=xh_sb[:, kd, :, w0:TBl],
                                start=False, stop=False, perf_mode=DR)
                        xl_sb = xls[b]
                        for kd in range(ND1):       # act-lo corr on [a0, TBl)
                            nc.tensor.matmul(
                                p1[:, a0:TBl],
                                lhsT=w1_ap(w1h_t, kd, fq0, fq1),
                                rhs=xl_sb[:, kd, :, a0:TBl],
                                start=False, stop=(kd == ND1 - 1),
                                perf_mode=DR)
                    hh_sb, hl_sb = hs[b]
                    kp, tp = i // 2, i % 2
                    hbf = hbpool.tile([P, 384], bf16, tag="hb", name="hb")
                    hb = hbf[:, :TBl]
                    nc.scalar.activation(hb[:], p1[:], Gelu,
                                         bias=b1_sb[:, i:i + 1],
                                         scale=inv_S1)
                    nc.vector.tensor_copy(hh_sb[kp][:, tp, :], hb[:])
                    if a0 < TBl:
                        nc.vector.tensor_sub(hl_sb[kp][:, tp, a0:TBl],
                                             hb[:, a0:TBl],
                                             hh_sb[kp][:, tp, a0:TBl])
                j = g - la
                if j >= 0 and j % 2 == 1:
                    jb, ji = divmod(j, KO2)
                    TBl = blocks[jb]
                    t0 = tok0s[jb]
                    TS = TBl // P
                    kd = ji // 2
                    if kd == 0:
                        ys[jb] = ypool.tile([P, TS, D], bf16, tag="y",
                                            name="y_sb")
                    hh_sb, hl_sb = hs[jb]
                    y_sb = ys[jb]
                    mo0 = t0 // P
                    last_b = (jb == NBLK - 1)
                    for ts in range(TS):
                        tier = btiers[jb][ts]
                        terms2 = [(0, w2h_t)]
                        if tier >= 2:
                            terms2.append((0, w2l_t))
                        if tier == 3:
                            terms2.append((1, w2h_t))
                        for t, (which, wp) in enumerate(terms2):
                            ha = (hh_sb if which == 0 else hl_sb)[kd]
                            for dh in range(DH):
                                nc.tensor.matmul(
                                    ypsum[ts][dh][:],
                                    lhsT=ha[:, :, ts * P:(ts + 1) * P],
                                    rhs=w2_ap(wp, kd, dh * DHW,
                                              (dh + 1) * DHW),
                                    start=(kd == 0 and t == 0),
                                    stop=(kd == ND2 - 1
                                          and t == len(terms2) - 1),
                                    perf_mode=DR,
                                )
                        if kd == ND2 - 1:
                            # evict this ts immediately; the next ts's final
                            # matmuls overlap the eviction chain
                            g_ap = gates_sb[:, mo0 + ts: mo0 + ts + 1]
                            nc.vector.tensor_scalar_mul(
                                y_sb[:, ts, 0:DHW], ypsum[ts][0][:], g_ap)
                            nc.scalar.activation(
                                y_sb[:, ts, DHW:D], ypsum[ts][1][:],
                                Copy, scale=g_ap)
                            r0 = t0 + ts * P
                            if last_b:
                                # SP HWDGE: lowest-latency path for the tail
                                nc.sync.dma_start(
                                    y[r0:r0 + P, :], y_sb[:, ts, :])
                            else:
                                nc.gpsimd.dma_start(
                                    y[r0:r0 + P, :], y_sb[:, ts, :])
    nc.compile()
    return nc


def kernel(x, Wg, bg, W1, b1, W2, b2):
    from concourse.bass_utils import run_bass_kernel_spmd

    x = np.asarray(x, dtype=np.float32)
    Wg = np.asarray(Wg, dtype=np.float32)
    bg = np.asarray(bg, dtype=np.float32)
    W1 = np.asarray(W1, dtype=np.float32)
    b1 = np.asarray(b1, dtype=np.float32)
    W2 = np.asarray(W2, dtype=np.float32)
    b2 = np.asarray(b2, dtype=np.float32)

    x_flat = x.reshape(-1, D)
    top2, g2 = _route(x_flat, Wg, bg)

    # Dispatch: per-expert token lists sorted by gate (descending)
    idx_e, gate_e = [], []
    for e in range(E):
        sel = np.nonzero(top2 == e)
        ids = sel[0].astype(np.int64)
        gs = g2[sel[0], sel[1]].astype(np.float32)
        order = np.argsort(-gs, kind="stable")
        idx_e.append(ids[order])
        gate_e.append(gs[order])
    counts = [len(i) for i in idx_e]
    C = CAP

    if C not in _nc_cache:
        _nc_cache[C] = _build_nc(C)
    nc = _nc_cache[C]

    blocks = BLOCKS
    ND1 = D // 256

    def x_block_pack(xp):
        segs = []
        t0 = 0
        for TBl in blocks:
            segs.append(xp[:, :, :, t0:t0 + TBl].reshape(P, -1))
            t0 += TBl
        return np.ascontiguousarray(np.concatenate(segs, axis=1))

    b1_packed_base = b1.reshape(E, F // P, P).transpose(0, 2, 1)  # [E, P, 24]

    in_maps = []
    for e in range(E):
        n_e = min(counts[e], C)
        assert n_e == C, "expert load below capacity; padding not needed here"
        # device slot s holds the dispatch of gate-rank _SLOT_RANKS[s]
        dev_ids = idx_e[e][_SLOT_RANKS]
        dev_gs = gate_e[e][_SLOT_RANKS]
        xTe = np.ascontiguousarray(x_flat[dev_ids].T)
        xh_p, xl_p = _hilo_pack(xTe, SX)
        w1h_p, w1l_p = _hilo_pack(W1[e], SW1)
        w2h_p, w2l_p = _hilo_pack(W2[e], SW2)
        ge = dev_gs / np.float32(SW2)
        in_maps.append({
            "xh": x_block_pack(xh_p), "xl": x_block_pack(xl_p),
            "w1h": w1h_p, "w1l": w1l_p,
            "w2h": w2h_p, "w2l": w2l_p,
            "b1": np.ascontiguousarray(b1_packed_base[e]),
            "gates": np.ascontiguousarray(
                ge.reshape(C // P, P).T),           # [P, C//P]
        })

    res = run_bass_kernel_spmd(nc, in_maps, core_ids=list(range(NCORES)))

    out = np.zeros((N, D), dtype=np.float32)
    for e in range(E):
        n_e = min(counts[e], C)
        dev_ids = idx_e[e][_SLOT_RANKS]
        out[dev_ids] += res.results[e]["y"].astype(np.float32)
        if counts[e] > n_e:
            # exact fp32 host path for capacity-overflow tokens (b2 is added
            # via the separable term below, so omit it here)
            oi = idx_e[e][n_e:]
            og = gate_e[e][n_e:]
            z = x_flat[oi] @ W1[e] + b1[e]
            h = 0.5 * z * (1.0 + _erf(z / np.float32(np.sqrt(2.0))))
            out[oi] += og[:, None] * (h @ W2[e])
    # separable b2 term: sum_k gate_k * b2[e_k]
    if np.any(b2):
        out += g2[:, 0:1] * b2[top2[:, 0]] + g2[:, 1:2] * b2[top2[:, 1]]
    return out.reshape(B, T, D)
